# revision 41
# baseline (speedup 1.0000x reference)
"""LoRA MultiheadAttention on 8 NeuronCores (Bass/Tile).

Sharding: 32 (batch, head) attention slices -> 4 heads x 1 batch per core.
Cores 0-3 take batch 0, cores 4-7 batch 1; core c handles heads
(c%4)*4 .. (c%4)*4+3, i.e. a contiguous 256-wide slice of the head dims.

Per-core math (all big matmuls bf16 on PE, fp32 PSUM accumulate):
  qkT   = wqk^T-slices @ X  -> Q^T, K^T in (head-dim, T) layout
          (q pre-scaled by 1/sqrt(hd)); Q bias added as a per-partition
          tensor_scalar during the PSUM->SBUF copy, K bias folded into a
          17th row of the LoRA-K accumulation matmul (ones row in A^T)
  V     = X @ Wv-slice, per-head 65-wide blocks with a ones column ->
          the PV matmul emits the softmax denominator for free; V bias +
          the ones-column constant folded into a 17th LoRA-V row
  S^T   = K^T.T-slices @ Q^T  (tj on partitions, ti free)  [K=64].
          Units are (head-pair x q-quarter): the pair's two K=64 matmuls
          run concurrently in PE row groups 0-63/64-127 (tile_position),
          halving S stream time into the two halves of one (128,1024)
          PSUM tile
  P^T   = exp(S^T) on ACT, 1024-wide tiles (no max-subtraction: |s|<~3)
  O^T   = V_aug.T @ P^T accumulated over tj; row 64 = denom
  norm  : units 0-6: denom row -> DRAM -> (64,16) reshape -> cheap
          InstReciprocal -> DRAM -> (64,1024) stride-0 broadcast ->
          multiply (split copy/finish so the single po bank pair frees
          before the next unit's PV).  Last unit: two 512-wide chains
          with 1/Z = exp(-ln Z) on the then-idle ACT (same table set as
          exp) and a K=1 ones-column matmul broadcast -- no DMA hops on
          the critical tail path.
  out   = O^T.T @ out_w-slice^T, bf16 partials summed on host.

The attention phase is ACT-exp-bound (16.8M exps/core = 109us floor
at 1 elem/lane/cycle @1.2GHz); with pair-packed S the PE has slack in
every unit, so Phase B(m1,m3) + all of Phase C + most of the out-proj
are woven into the attention units' PE stream as fillers: ACT runs
continuously while the PE never gaps >3.4us (keeps HAM at 2.4 GHz).

out_b added on host.
"""

import sys

sys.path.insert(0, "/opt/trn_rl_repo")

import math
from contextlib import ExitStack

import ml_dtypes
import numpy as np

import concourse.bass as bass
import concourse.tile as tile
from concourse import bacc
from concourse import mybir
from concourse import bass_utils as _bu
from concourse.bass_utils import run_bass_kernel_spmd

# (note: walrus's --enable-ldw-opt is hardcoded false for a reason --
# bass-emitted InstLdweights is rejected by that pass.)

BF16 = ml_dtypes.bfloat16
F32 = mybir.dt.float32
BF = mybir.dt.bfloat16

T = 2048
D = 1024
H = 16
HD = 64
R = 16
RA = R + 1  # LoRA rank + ones row (bias folding)
BSZ = 2
SCALE = 16.0
NCORES = 8
HPC = 4  # heads per core
CD = HPC * HD  # 256 head dims per core
VW = HD + 1  # V block width per head (ones column appended)
NKT = D // 128  # 8 contraction k-tiles (no bias row: biases are folded)
P = 128
NTT = T // P  # 16 row tiles
HF = T // 2  # 1024: ti processed in two halves


def build_nc():
    nc = bass.Bass()
    xa = nc.dram_tensor("xa", [D, T], BF, kind="ExternalInput")
    wqk = nc.dram_tensor("wqk", [D, 2 * CD], BF, kind="ExternalInput")
    wv = nc.dram_tensor("wv", [P, NKT * HPC * VW], BF, kind="ExternalInput")
    ab = nc.dram_tensor("ab", [P, NKT * 3 * R], BF, kind="ExternalInput")
    kbm = nc.dram_tensor("kbm", [RA, CD], BF, kind="ExternalInput")
    vbm = nc.dram_tensor("vbm", [RA, HPC * VW], BF, kind="ExternalInput")
    qb = nc.dram_tensor("qb", [P, 2], F32, kind="ExternalInput")
    wo = nc.dram_tensor("wo", [CD, D], BF, kind="ExternalInput")
    out = nc.dram_tensor("out", [T, D], BF, kind="ExternalOutput")

    with tile.TileContext(nc) as tc, ExitStack() as ctx:
        singles = ctx.enter_context(tc.tile_pool(name="singles", bufs=1))

        xa_t = [singles.tile([P, T], BF, name=f"xa{i}", tag=f"xa{i}") for i in range(NKT)]
        wqk_t = [singles.tile([P, 2 * CD], BF, name=f"wqk{i}", tag=f"wqk{i}") for i in range(NKT)]
        # ab/wv k-tiles packed column-wise into one tile each: their
        # natural per-tile partition rows are 96/520 bytes, which makes the
        # DMA descriptor-rate-bound (1024 tiny descriptors held Phase A's
        # weights hostage for ~12us).  Packed, the rows are 8x larger.
        wvp = singles.tile([P, NKT * HPC * VW], BF, tag="wvp")
        abp = singles.tile([P, NKT * 3 * R], BF, tag="abp")
        wv_t = [wvp[:, i * HPC * VW : (i + 1) * HPC * VW] for i in range(NKT)]
        ab_t = [abp[:, i * 3 * R : (i + 1) * 3 * R] for i in range(NKT)]
        kb_t = singles.tile([RA, CD], BF, tag="kb")
        vb_t = singles.tile([RA, HPC * VW], BF, tag="vb")
        qb_t = singles.tile([P, 2], F32, tag="qb")
        wo_t = [singles.tile([P, D], BF, name=f"wo{i}", tag=f"wo{i}") for i in range(2)]
        # DMA issue order matches consumption: A needs ab+xa, then B needs
        # wqk, C (interleaved into unit 0) needs wv, out-proj needs wo last.
        # Input DMAs spread across three engine queues: ~600ns of submit
        # cost per DMA serializes on a single queue (27 submits = 16us
        # before the last transfer even starts), and the transfers
        # themselves overlap across hardware queues.  Order within each
        # queue matches consumption: A needs ab+xa first, B m0 needs wqk
        # ~10us in, C fillers need wv by ~unit 0, wo last.
        for i in range(0, NKT, 2):
            nc.sync.dma_start(out=xa_t[i], in_=xa[i * P : (i + 1) * P, :])
            nc.scalar.dma_start(out=xa_t[i + 1], in_=xa[(i + 1) * P : (i + 2) * P, :])
            nc.scalar.dma_start(out=wqk_t[i], in_=wqk[i * P : (i + 1) * P, :])
            nc.sync.dma_start(out=wqk_t[i + 1], in_=wqk[(i + 1) * P : (i + 2) * P, :])
        nc.sync.dma_start(out=qb_t, in_=qb[:, :])
        nc.sync.dma_start(out=kb_t, in_=kbm[:, :])
        nc.scalar.dma_start(out=abp, in_=ab[:, :])
        nc.scalar.dma_start(out=vb_t, in_=vbm[:, :])
        nc.sync.dma_start(out=wvp, in_=wv[:, :])
        for i in range(2):
            nc.scalar.dma_start(out=wo_t[i], in_=wo[i * P : (i + 1) * P, :])

        ones_t = singles.tile([1, HD], F32, tag="ones")
        nc.vector.memset(ones_t, 1.0)

        # Warm-up: the PE sits idle ~7-15us waiting for the first xa/wqk
        # transfers, and HAM only promotes to 2.4 GHz after ~3.4us of
        # sustained activity -- so the early prologue matmuls would run at
        # half clock.  A stream of dep-free K=1 dummy matmuls (~60ns each)
        # occupies the PE through the DMA ramp: HAM is warm before the
        # first real matmul issues.
        with tc.tile_pool(name="pW", bufs=1, space="PSUM") as pW:
            wrm = pW.tile([HD, HD], F32, tag="wrm", name="wrm")
            for _ in range(160):
                nc.tensor.matmul(wrm, lhsT=ones_t, rhs=ones_t, start=True, stop=True)

        # Dummy exp with no deps: walrus's ACT_TABLE_LOAD for the exp set
        # (~2.7us) runs during the input DMA wait instead of at the first
        # real attention exp.
        scr_t = singles.tile([1, HD], BF, tag="scr")
        nc.scalar.activation(scr_t, ones_t, mybir.ActivationFunctionType.Exp)

        qk_sb = [singles.tile([P, T], BF, name=f"qk{i}", tag=f"qk{i}") for i in range(4)]
        ak_sb = singles.tile([RA, T], BF, tag="ak")
        av_sb = singles.tile([RA, T], BF, tag="av")
        v_sb = [singles.tile([P, HPC * VW], BF, name=f"v{i}", tag=f"v{i}") for i in range(NTT)]
        oT_sb = [singles.tile([P, T], BF, name=f"oT{i}", tag=f"oT{i}") for i in range(2)]

        # ones row for the bias-folding contraction (row 16 of A^T tiles):
        # engines can't address a 1-partition region at base 16, so memset
        # the whole tile and let Phase A overwrite rows 0-15.
        nc.vector.memset(ak_sb, 1.0)
        nc.vector.memset(av_sb, 1.0)

        # Prologue: B m0 and B m2's main K=1024 contractions ride one
        # kt-outer loop (8 live accumulators = all 8 PSUM banks), paced by
        # the two DMA queues delivering xa[kt]+wqk[kt].  Phase A then runs
        # on the 4 banks freed by the m0 copies (same-tag tile reuse), and
        # the LoRA-K + K-bias matmul accumulates into the still-live m2
        # banks before their copies.  Serial pre-attention work after the
        # input stream shrinks to A + lora + copies (~9us).
        with tc.tile_pool(name="pPro", bufs=1, space="PSUM") as pPro:
            pqs = [
                pPro.tile([P, 512], F32, tag=f"pq{ch}", name=f"pq0{ch}")
                for ch in range(4)
            ]
            pq2s = [
                pPro.tile([P, 512], F32, tag=f"pq2{ch}", name=f"pq2{ch}")
                for ch in range(4)
            ]
            for kt in range(NKT):
                for ch in range(4):
                    nc.tensor.matmul(
                        pqs[ch],
                        lhsT=wqk_t[kt][:, 0:P],
                        rhs=xa_t[kt][:, ch * 512 : (ch + 1) * 512],
                        start=(kt == 0),
                        stop=(kt == NKT - 1),
                    )
                for ch in range(4):
                    nc.tensor.matmul(
                        pq2s[ch],
                        lhsT=wqk_t[kt][:, 2 * P : 3 * P],
                        rhs=xa_t[kt][:, ch * 512 : (ch + 1) * 512],
                        start=(kt == 0),
                        stop=False,
                    )
            for ch in range(4):
                cs = slice(ch * 512, (ch + 1) * 512)
                nc.vector.tensor_scalar_add(qk_sb[0][:, cs], pqs[ch], qb_t[:, 0:1])
            # Phase A on the freed m0 banks, with the LoRA-K + K-bias
            # matmul and m2 copy of chunk ch-1 interleaved after A's chunk
            # ch so the PE reaches the first attention S matmul with its
            # qk_sb[2] dependency already satisfied (no end-of-prologue
            # bubble, no HAM re-throttle at attention start).
            def lora_m2(ch):
                cs = slice(ch * 512, (ch + 1) * 512)
                nc.tensor.matmul(
                    pq2s[ch], lhsT=kb_t[:, 0:P], rhs=ak_sb[:, cs],
                    start=False, stop=True,
                )
                nc.vector.tensor_copy(qk_sb[2][:, cs], pq2s[ch])

            for ch in range(4):
                cs = slice(ch * 512, (ch + 1) * 512)
                pa = pPro.tile([3 * R, 512], F32, tag=f"pq{ch}", name=f"pa{ch}")
                for kt in range(NKT):
                    nc.tensor.matmul(
                        pa,
                        lhsT=ab_t[kt],
                        rhs=xa_t[kt][:, cs],
                        start=(kt == 0),
                        stop=(kt == NKT - 1),
                    )
                nc.vector.tensor_copy(ak_sb[0:R, cs], pa[0:R, :])
                nc.vector.tensor_copy(av_sb[0:R, cs], pa[2 * R : 3 * R, :])
                if ch >= 1:
                    lora_m2(ch - 1)
            lora_m2(3)

        # Phase D+E: attention units restructured as (head-pair x
        # q-quarter): the two heads' S^T matmuls have K=64 and run
        # CONCURRENTLY in row groups 0-63 / 64-127 (tile_position derives
        # from the qk_sb base partitions), writing the two bank-halves of
        # one (128,1024) PSUM tile -- S stream time halves vs one-head
        # units while the 1024-wide exp and PSUM budget stay identical.
        # Per tj the PE now does ~0.64us (S pair ~0.21 + 2 PV 0.43) vs
        # ACT's 1.15us exp, so every unit has slack to absorb the woven-in
        # B/C/out-proj fillers.  PSUM (8 banks): pS 2x(128,1024)=4,
        # pO 2x(65,512)=2, pX 2x(128,512)=2.
        with (
            tc.tile_pool(name="pS", bufs=2, space="PSUM") as pS,
            tc.tile_pool(name="pO", bufs=2, space="PSUM") as pO,
            tc.tile_pool(name="pX", bufs=2, space="PSUM") as pX,
            tc.tile_pool(name="pP", bufs=3) as pP,
            tc.tile_pool(name="pN", bufs=4) as pN,
            tc.tile_pool(name="pD", bufs=4, space="DRAM") as pD,
            tc.tile_pool(name="pOut", bufs=3) as pOut,
        ):
            def filler_c(mt):
                def f():
                    ms = slice(mt * P, (mt + 1) * P)
                    pv = pX.tile([P, 512], F32, tag="px", name=f"pv_{mt}")
                    for kt in range(NKT):
                        nc.tensor.matmul(
                            pv[:, : HPC * VW],
                            lhsT=xa_t[kt][:, ms],
                            rhs=wv_t[kt],
                            start=(kt == 0),
                            stop=False,
                        )
                    nc.tensor.matmul(
                        pv[:, : HPC * VW], lhsT=av_sb[:, ms], rhs=vb_t,
                        start=False, stop=True,
                    )
                    nc.vector.tensor_copy(v_sb[mt], pv[:, : HPC * VW])
                return f

            def filler_b(m, ch):
                def f():
                    cs = slice(ch * 512, (ch + 1) * 512)
                    pq = pX.tile([P, 512], F32, tag="px", name=f"pq_{m}_{ch}")
                    for kt in range(NKT):
                        nc.tensor.matmul(
                            pq,
                            lhsT=wqk_t[kt][:, m * P : (m + 1) * P],
                            rhs=xa_t[kt][:, cs],
                            start=(kt == 0),
                            stop=(kt == NKT - 1 and m < 2),
                        )
                    if m >= 2:
                        nc.tensor.matmul(
                            pq,
                            lhsT=kb_t[:, (m - 2) * P : (m - 1) * P],
                            rhs=ak_sb[:, cs],
                            start=False,
                            stop=True,
                        )
                        nc.vector.tensor_copy(qk_sb[m][:, cs], pq)
                    else:
                        nc.vector.tensor_scalar_add(
                            qk_sb[m][:, cs], pq, qb_t[:, m : m + 1]
                        )
                return f

            def outproj_mt(mt, act_copies):
                def f():
                    ms = slice(mt * P, (mt + 1) * P)
                    ob = pOut.tile([P, D], BF, tag="ob", name=f"ob_{mt}")
                    for ch in range(2):
                        cs = slice(ch * 512, (ch + 1) * 512)
                        px = pX.tile([P, 512], F32, tag="px", name=f"px_{mt}_{ch}")
                        for kt2 in range(2):
                            nc.tensor.matmul(
                                px,
                                lhsT=oT_sb[kt2][:, ms],
                                rhs=wo_t[kt2][:, cs],
                                start=(kt2 == 0),
                                stop=(kt2 == 1),
                            )
                        if act_copies and ch == 1:
                            nc.scalar.copy(ob[:, cs], px)
                        else:
                            nc.vector.tensor_copy(ob[:, cs], px)
                        nc.sync.dma_start(out=out[ms, cs], in_=ob[:, cs])
                return f

            def emit_unit(pair, qq, fillers, stride, seam=None):
                qTt = qk_sb[pair]
                kTt = qk_sb[2 + pair]
                qs = slice(qq * 512, (qq + 1) * 512)
                pos = [
                    pO.tile([VW, 512], F32, tag="po", name=f"po_{pair}_{qq}_{hh}")
                    for hh in range(2)
                ]
                pts = {}

                def emit_pv(tjp):
                    pt = pts.pop(tjp)
                    for hh in range(2):
                        h = 2 * pair + hh
                        nc.tensor.matmul(
                            pos[hh],
                            lhsT=v_sb[tjp][:, h * VW : (h + 1) * VW],
                            rhs=pt[:, hh * 512 : (hh + 1) * 512],
                            start=(tjp == 0),
                            stop=(tjp == NTT - 1),
                        )

                for tj in range(NTT):
                    ps = pS.tile([P, HF], F32, tag="spsum", name=f"ps_{pair}_{qq}_{tj}")
                    for hh in range(2):
                        nc.tensor.matmul(
                            ps[:, hh * 512 : (hh + 1) * 512],
                            lhsT=kTt[hh * HD : (hh + 1) * HD, tj * P : (tj + 1) * P],
                            rhs=qTt[hh * HD : (hh + 1) * HD, qs],
                            start=True,
                            stop=True,
                        )
                    pt = pP.tile([P, HF], BF, tag="pt", name=f"pt_{pair}_{qq}_{tj}")
                    nc.scalar.activation(pt, ps, mybir.ActivationFunctionType.Exp)
                    pts[tj] = pt
                    if tj == 0 and seam is not None:
                        seam()
                    if fillers and tj % stride == 0:
                        fillers.pop(0)()
                    if tj > 0:
                        emit_pv(tj - 1)
                return pos, lambda: emit_pv(NTT - 1)

            def emit_norm_copy(pair, qq, hh, po):
                un = pN.tile([VW, 512], F32, tag="un", name=f"un_{pair}_{qq}_{hh}")
                nc.vector.tensor_copy(un, po)
                dr = pD.tile([1, 512], F32, tag="dr", name=f"dr_{pair}_{qq}_{hh}")
                nc.sync.dma_start(out=dr, in_=un[HD:VW, :])
                rs = pN.tile([HD, 8], F32, tag="rs", name=f"rs_{pair}_{qq}_{hh}")
                nc.sync.dma_start(
                    out=rs,
                    in_=bass.AP(tensor=dr.tensor, offset=dr.offset, ap=[[8, HD], [1, 8]]),
                )
                rr = pN.tile([HD, 8], F32, tag="rr", name=f"rr_{pair}_{qq}_{hh}")
                nc.vector.reciprocal(rr, rs)
                dr2 = pD.tile([HD, 8], F32, tag="dr2", name=f"dr2_{pair}_{qq}_{hh}")
                nc.sync.dma_start(out=dr2, in_=rr)
                rec = pN.tile([HD, 512], F32, tag="rec", name=f"rec_{pair}_{qq}_{hh}")
                nc.sync.dma_start(
                    out=rec,
                    in_=bass.AP(tensor=dr2.tensor, offset=dr2.offset, ap=[[0, HD], [1, 512]]),
                )
                return un, rec

            def emit_norm_finish(pair, qq, hh, un, rec):
                nc.vector.tensor_mul(
                    oT_sb[pair][hh * HD : (hh + 1) * HD, qq * 512 : (qq + 1) * 512],
                    un[0:HD, :],
                    rec,
                )

            def emit_tail(pair, qq, pos):
                # Last unit's two heads normalized via 1/Z = exp(-ln Z) on
                # the now-idle ACT (same table set as exp) + a K=1
                # ones-column matmul broadcast -- no DMA on the tail path;
                # dummy matmuls off the chain tiles keep HAM warm.
                for hh in range(2):
                    po = pos[hh]
                    un = pN.tile([VW, 512], F32, tag="unc", name=f"unc_{hh}")
                    nc.vector.tensor_copy(un, po)
                    lnz = pN.tile([1, 512], F32, tag="lnz", name=f"lnz_{hh}")
                    nc.scalar.activation(
                        lnz, un[HD:VW, :], mybir.ActivationFunctionType.Ln
                    )
                    rcp = pN.tile([1, 512], F32, tag="rcp", name=f"rcp_{hh}")
                    nc.scalar.activation(
                        rcp, lnz, mybir.ActivationFunctionType.Exp, scale=-1.0
                    )
                    dm = pS.tile([P, HF], F32, tag="spsum", name=f"dm_a{hh}")
                    nc.tensor.matmul(
                        dm[:, 0:64], lhsT=un[0:HD, 0:P], rhs=un[0:HD, 0:64],
                        start=True, stop=True,
                    )
                    rec = pX.tile([P, 512], F32, tag="px", name=f"recp_{hh}")
                    nc.tensor.matmul(
                        rec[0:HD, :], lhsT=ones_t, rhs=rcp, start=True, stop=True
                    )
                    nc.vector.tensor_mul(
                        oT_sb[pair][hh * HD : (hh + 1) * HD, qq * 512 : (qq + 1) * 512],
                        un[0:HD, :],
                        rec[0:HD, :],
                    )
                for k, mt in enumerate(range(12, 16)):
                    outproj_mt(mt, act_copies=(k % 2 == 0))()

            # Units: pair-01 quarters then pair-23 quarters.  C mt j must be
            # emitted by unit-0 slot j (PV consumes v_sb[j]); B m1/m3 must
            # finish before unit 4 (pair 23) -> spread over units 1-3;
            # out-proj group g (mt 4g..4g+3) unlocks when unit 4+g's norm
            # finishes (seam of unit 6+g) -> groups 0,1 woven into units
            # 6,7 as fillers, group 2 post-loop, group 3 after the tail.
            fillers_u = [[] for _ in range(8)]
            fillers_u[0] = [filler_c(mt) for mt in range(NTT)]
            bl = [filler_b(m, ch) for ch in range(4) for m in (1, 3)]
            fillers_u[1] = bl[0:3]
            fillers_u[2] = bl[3:6]
            fillers_u[3] = bl[6:8]
            strides = [1, 5, 5, 5, 4, 4, 4, 4]

            # bridge the ~1.7us dep-wait before unit 0's first S matmul
            # (m2-ch0 copy in flight) so HAM doesn't re-throttle right at
            # attention start
            dmw = pS.tile([P, HF], F32, tag="spsum", name="dmw")
            for _ in range(30):
                nc.tensor.matmul(
                    dmw[0:HD, 0:HD], lhsT=ones_t, rhs=ones_t, start=True, stop=True
                )

            units = [(pair, qq) for pair in range(2) for qq in range(4)]
            state = {"prev": None, "tofinish": None}

            def make_seam(i):
                def seam():
                    p = state["prev"]
                    if p is None:
                        return
                    p["final_pv"]()
                    cur = None
                    if p["idx"] < 7:
                        cur = [
                            emit_norm_copy(p["pair"], p["qq"], hh, p["pos"][hh])
                            for hh in range(2)
                        ]
                    f = state["tofinish"]
                    if f is not None:
                        for args in f:
                            emit_norm_finish(*args)
                        g = i - 6
                        if g >= 0:
                            for j in range(4):
                                fillers_u[i].append(
                                    outproj_mt(4 * g + j, act_copies=False)
                                )
                    state["tofinish"] = (
                        [
                            (p["pair"], p["qq"], hh, cur[hh][0], cur[hh][1])
                            for hh in range(2)
                        ]
                        if cur
                        else None
                    )
                return seam

            for i, (pair, qq) in enumerate(units):
                pos, fpv = emit_unit(
                    pair, qq, fillers_u[i], strides[i], make_seam(i)
                )
                state["prev"] = {
                    "idx": i, "pair": pair, "qq": qq, "pos": pos, "final_pv": fpv,
                }
            state["prev"]["final_pv"]()
            for args in state["tofinish"]:
                emit_norm_finish(*args)
            for j in range(8, 12):
                outproj_mt(j, act_copies=False)()
            emit_tail(1, 3, state["prev"]["pos"])

    # bass.Bass's finalize skips Bacc's wait-splitting passes; walrus allows
    # at most 1 sync wait per instruction (2 for event semaphores), so run
    # just those two passes here.
    import bass_rust as _bass_rust

    _bass_rust.move_matmul_waits_to_ldweights(nc.m)
    _bass_rust.generate_event_semaphores(nc)
    return nc


def prepare_in_maps(inputs):
    q = np.asarray(inputs["query"], np.float32)
    ipw = np.asarray(inputs["in_proj_weight"], np.float32)
    ipb = np.asarray(inputs["in_proj_bias"], np.float32)
    out_w = np.asarray(inputs["out_w"], np.float32)
    k_a = np.asarray(inputs["k_a"], np.float32)
    k_b = np.asarray(inputs["k_b"], np.float32)
    v_a = np.asarray(inputs["v_a"], np.float32)
    v_b = np.asarray(inputs["v_b"], np.float32)
    qscale = 1.0 / math.sqrt(HD)
    sl = SCALE / R

    in_maps = []
    for c in range(NCORES):
        bb = c // 4
        s = (c % 4) * CD
        e = s + CD
        X = q[:, bb, :]

        xa = X.T  # (D, T)

        wqk = np.zeros((D, 2 * CD), np.float32)
        wqk[:, :CD] = ipw[s:e].T * qscale
        wqk[:, CD:] = ipw[D + s : D + e].T

        wv = np.zeros((D, HPC * VW), np.float32)
        for j in range(HPC):
            wv[:, j * VW : j * VW + HD] = ipw[2 * D + s + j * HD : 2 * D + s + (j + 1) * HD].T
        # pack 8 k-tiles column-wise (see kernel comment on DMA descriptors)
        wv = wv.reshape(NKT, P, HPC * VW).transpose(1, 0, 2).reshape(P, NKT * HPC * VW)

        ab = np.zeros((D, 3 * R), np.float32)
        ab[:, :R] = k_a.T
        ab[:, 2 * R :] = v_a.T
        ab = ab.reshape(NKT, P, 3 * R).transpose(1, 0, 2).reshape(P, NKT * 3 * R)

        kbm = np.zeros((RA, CD), np.float32)
        kbm[:R] = k_b[:, s:e] * sl
        kbm[R] = ipb[D + s : D + e]  # K bias via ones row

        vbm = np.zeros((RA, HPC * VW), np.float32)
        for j in range(HPC):
            vbm[:R, j * VW : j * VW + HD] = v_b[:, s + j * HD : s + (j + 1) * HD] * sl
            vbm[R, j * VW : j * VW + HD] = ipb[2 * D + s + j * HD : 2 * D + s + (j + 1) * HD]
            vbm[R, j * VW + HD] = 1.0  # denominator ones column

        qbias = np.stack([ipb[s : s + P], ipb[s + P : s + 2 * P]], axis=1) * qscale

        wo = out_w[:, s:e].T

        in_maps.append(
            {
                "xa": xa.astype(BF16),
                "wqk": wqk.astype(BF16),
                "wv": wv.astype(BF16),
                "ab": ab.astype(BF16),
                "kbm": kbm.astype(BF16),
                "vbm": vbm.astype(BF16),
                "qb": qbias.astype(np.float32),
                "wo": wo.astype(BF16),
            }
        )
    return in_maps


def assemble_output(inputs, results):
    out_b = np.asarray(inputs["out_b"], np.float32)
    out = np.zeros((T, BSZ, D), np.float32)
    for c in range(NCORES):
        out[:, c // 4, :] += results[c]["out"].astype(np.float32)
    out += out_b[None, None, :]
    return out


def kernel(**inputs):
    nc = build_nc()
    in_maps = prepare_in_maps(inputs)
    res = run_bass_kernel_spmd(nc, in_maps, core_ids=list(range(NCORES)))
    return assemble_output(inputs, res.results)


# revision 42
# speedup vs baseline: 1.1355x; 1.1355x over previous
"""LoRA MultiheadAttention on 8 NeuronCores (Bass/Tile).

Sharding: 32 (batch, head) attention slices -> 4 heads x 1 batch per core.
Cores 0-3 take batch 0, cores 4-7 batch 1; core c handles heads
(c%4)*4 .. (c%4)*4+3, i.e. a contiguous 256-wide slice of the head dims.

Per-core math (all big matmuls bf16 on PE, fp32 PSUM accumulate):
  qkT   = wqk^T-slices @ X  -> Q^T, K^T in (head-dim, T) layout
          (q pre-scaled by 1/sqrt(hd)); Q bias added as a per-partition
          tensor_scalar during the PSUM->SBUF copy, K bias folded into a
          17th row of the LoRA-K accumulation matmul (ones row in A^T)
  V     = X @ Wv-slice, per-head 65-wide blocks with a ones column ->
          the PV matmul emits the softmax denominator for free; V bias +
          the ones-column constant folded into a 17th LoRA-V row
  S^T   = K^T.T-slices @ Q^T  (tj on partitions, ti free)  [K=64].
          Units are (head-pair x q-quarter): the pair's two K=64 matmuls
          run concurrently in PE row groups 0-63/64-127 (tile_position),
          halving S stream time into the two halves of one (128,1024)
          PSUM tile
  P^T   = exp(S^T) on ACT, 1024-wide tiles (no max-subtraction: |s|<~3)
  O^T   = V_aug.T @ P^T accumulated over tj; row 64 = denom
  norm  : units 0-6: denom row -> DRAM -> (64,16) reshape -> cheap
          InstReciprocal -> DRAM -> (64,1024) stride-0 broadcast ->
          multiply (split copy/finish so the single po bank pair frees
          before the next unit's PV).  Last unit: two 512-wide chains
          with 1/Z = exp(-ln Z) on the then-idle ACT (same table set as
          exp) and a K=1 ones-column matmul broadcast -- no DMA hops on
          the critical tail path.
  out   = O^T.T @ out_w-slice^T, bf16 partials summed on host.

The attention phase is ACT-exp-bound (16.8M exps/core = 109us floor
at 1 elem/lane/cycle @1.2GHz); with pair-packed S the PE has slack in
every unit, so Phase B(m1,m3) + all of Phase C + most of the out-proj
are woven into the attention units' PE stream as fillers: ACT runs
continuously while the PE never gaps >3.4us (keeps HAM at 2.4 GHz).

out_b added on host.
"""

import sys

sys.path.insert(0, "/opt/trn_rl_repo")

import math
from contextlib import ExitStack

import ml_dtypes
import numpy as np

import concourse.bass as bass
import concourse.tile as tile
from concourse import bacc
from concourse import mybir
from concourse import bass_utils as _bu
from concourse.bass_utils import run_bass_kernel_spmd

# (note: walrus's --enable-ldw-opt is hardcoded false for a reason --
# bass-emitted InstLdweights is rejected by that pass.)

BF16 = ml_dtypes.bfloat16
F32 = mybir.dt.float32
BF = mybir.dt.bfloat16

T = 2048
D = 1024
H = 16
HD = 64
R = 16
RA = R + 1  # LoRA rank + ones row (bias folding)
BSZ = 2
SCALE = 16.0
NCORES = 8
HPC = 4  # heads per core
CD = HPC * HD  # 256 head dims per core
VW = HD + 1  # V block width per head (ones column appended)
NKT = D // 128  # 8 contraction k-tiles (no bias row: biases are folded)
P = 128
NTT = T // P  # 16 row tiles
HF = T // 2  # 1024: ti processed in two halves


def build_nc():
    nc = bass.Bass()
    xa = nc.dram_tensor("xa", [D, T], BF, kind="ExternalInput")
    wqk = nc.dram_tensor("wqk", [D, 2 * CD], BF, kind="ExternalInput")
    wv = nc.dram_tensor("wv", [P, NKT * HPC * VW], BF, kind="ExternalInput")
    ab = nc.dram_tensor("ab", [P, NKT * 3 * R], BF, kind="ExternalInput")
    kbm = nc.dram_tensor("kbm", [RA, CD], BF, kind="ExternalInput")
    vbm = nc.dram_tensor("vbm", [RA, HPC * VW], BF, kind="ExternalInput")
    qb = nc.dram_tensor("qb", [P, 2], F32, kind="ExternalInput")
    wo = nc.dram_tensor("wo", [CD, D], BF, kind="ExternalInput")
    out = nc.dram_tensor("out", [T, D], BF, kind="ExternalOutput")

    with tile.TileContext(nc) as tc, ExitStack() as ctx:
        singles = ctx.enter_context(tc.tile_pool(name="singles", bufs=1))

        xa_t = [singles.tile([P, T], BF, name=f"xa{i}", tag=f"xa{i}") for i in range(NKT)]
        wqk_t = [singles.tile([P, 2 * CD], BF, name=f"wqk{i}", tag=f"wqk{i}") for i in range(NKT)]
        # ab/wv k-tiles packed column-wise into one tile each: their
        # natural per-tile partition rows are 96/520 bytes, which makes the
        # DMA descriptor-rate-bound (1024 tiny descriptors held Phase A's
        # weights hostage for ~12us).  Packed, the rows are 8x larger.
        wvp = singles.tile([P, NKT * HPC * VW], BF, tag="wvp")
        abp = singles.tile([P, NKT * 3 * R], BF, tag="abp")
        wv_t = [wvp[:, i * HPC * VW : (i + 1) * HPC * VW] for i in range(NKT)]
        ab_t = [abp[:, i * 3 * R : (i + 1) * 3 * R] for i in range(NKT)]
        kb_t = singles.tile([RA, CD], BF, tag="kb")
        vb_t = singles.tile([RA, HPC * VW], BF, tag="vb")
        qb_t = singles.tile([P, 2], F32, tag="qb")
        wo_t = [singles.tile([P, D], BF, name=f"wo{i}", tag=f"wo{i}") for i in range(2)]
        # DMA issue order matches consumption: A needs ab+xa, then B needs
        # wqk, C (interleaved into unit 0) needs wv, out-proj needs wo last.
        # Input DMAs spread across three engine queues: ~600ns of submit
        # cost per DMA serializes on a single queue (27 submits = 16us
        # before the last transfer even starts), and the transfers
        # themselves overlap across hardware queues.  Order within each
        # queue matches consumption: A needs ab+xa first, B m0 needs wqk
        # ~10us in, C fillers need wv by ~unit 0, wo last.
        for i in range(0, NKT, 2):
            nc.sync.dma_start(out=xa_t[i], in_=xa[i * P : (i + 1) * P, :])
            nc.scalar.dma_start(out=xa_t[i + 1], in_=xa[(i + 1) * P : (i + 2) * P, :])
            nc.scalar.dma_start(out=wqk_t[i], in_=wqk[i * P : (i + 1) * P, :])
            nc.sync.dma_start(out=wqk_t[i + 1], in_=wqk[(i + 1) * P : (i + 2) * P, :])
        nc.sync.dma_start(out=qb_t, in_=qb[:, :])
        nc.sync.dma_start(out=kb_t, in_=kbm[:, :])
        nc.scalar.dma_start(out=abp, in_=ab[:, :])
        nc.scalar.dma_start(out=vb_t, in_=vbm[:, :])
        nc.sync.dma_start(out=wvp, in_=wv[:, :])
        for i in range(2):
            nc.scalar.dma_start(out=wo_t[i], in_=wo[i * P : (i + 1) * P, :])

        ones_t = singles.tile([1, HD], F32, tag="ones")
        nc.vector.memset(ones_t, 1.0)

        # Warm-up: the PE sits idle ~7-15us waiting for the first xa/wqk
        # transfers, and HAM only promotes to 2.4 GHz after ~3.4us of
        # sustained activity -- so the early prologue matmuls would run at
        # half clock.  A stream of dep-free K=1 dummy matmuls (~60ns each)
        # occupies the PE through the DMA ramp: HAM is warm before the
        # first real matmul issues.
        with tc.tile_pool(name="pW", bufs=1, space="PSUM") as pW:
            wrm = pW.tile([HD, HD], F32, tag="wrm", name="wrm")
            for _ in range(55):
                nc.tensor.matmul(wrm, lhsT=ones_t, rhs=ones_t, start=True, stop=True)

        # Dummy exp with no deps: walrus's ACT_TABLE_LOAD for the exp set
        # (~2.7us) runs during the input DMA wait instead of at the first
        # real attention exp.
        scr_t = singles.tile([1, HD], BF, tag="scr")
        nc.scalar.activation(scr_t, ones_t, mybir.ActivationFunctionType.Exp)

        qk_sb = [singles.tile([P, T], BF, name=f"qk{i}", tag=f"qk{i}") for i in range(4)]
        ak_sb = singles.tile([RA, T], BF, tag="ak")
        av_sb = singles.tile([RA, T], BF, tag="av")
        v_sb = [singles.tile([P, HPC * VW], BF, name=f"v{i}", tag=f"v{i}") for i in range(NTT)]
        oT_sb = [singles.tile([P, T], BF, name=f"oT{i}", tag=f"oT{i}") for i in range(2)]

        # ones row for the bias-folding contraction (row 16 of A^T tiles):
        # engines can't address a 1-partition region at base 16, so memset
        # the whole tile and let Phase A overwrite rows 0-15.
        nc.vector.memset(ak_sb, 1.0)
        nc.vector.memset(av_sb, 1.0)

        # Prologue: B m0 and B m2's main K=1024 contractions ride one
        # kt-outer loop (8 live accumulators = all 8 PSUM banks), paced by
        # the two DMA queues delivering xa[kt]+wqk[kt].  Phase A then runs
        # on the 4 banks freed by the m0 copies (same-tag tile reuse), and
        # the LoRA-K + K-bias matmul accumulates into the still-live m2
        # banks before their copies.  Serial pre-attention work after the
        # input stream shrinks to A + lora + copies (~9us).
        with tc.tile_pool(name="pPro", bufs=1, space="PSUM") as pPro:
            pqs = [
                pPro.tile([P, 512], F32, tag=f"pq{ch}", name=f"pq0{ch}")
                for ch in range(4)
            ]
            pq2s = [
                pPro.tile([P, 512], F32, tag=f"pq2{ch}", name=f"pq2{ch}")
                for ch in range(4)
            ]
            for kt in range(NKT):
                for ch in range(4):
                    nc.tensor.matmul(
                        pqs[ch],
                        lhsT=wqk_t[kt][:, 0:P],
                        rhs=xa_t[kt][:, ch * 512 : (ch + 1) * 512],
                        start=(kt == 0),
                        stop=(kt == NKT - 1),
                    )
                for ch in range(4):
                    nc.tensor.matmul(
                        pq2s[ch],
                        lhsT=wqk_t[kt][:, 2 * P : 3 * P],
                        rhs=xa_t[kt][:, ch * 512 : (ch + 1) * 512],
                        start=(kt == 0),
                        stop=False,
                    )
            for ch in range(4):
                cs = slice(ch * 512, (ch + 1) * 512)
                nc.vector.tensor_scalar_add(qk_sb[0][:, cs], pqs[ch], qb_t[:, 0:1])
            # Phase A on the freed m0 banks, with the LoRA-K + K-bias
            # matmul and m2 copy of chunk ch-1 interleaved after A's chunk
            # ch so the PE reaches the first attention S matmul with its
            # qk_sb[2] dependency already satisfied (no end-of-prologue
            # bubble, no HAM re-throttle at attention start).
            def lora_m2(ch):
                cs = slice(ch * 512, (ch + 1) * 512)
                nc.tensor.matmul(
                    pq2s[ch], lhsT=kb_t[:, 0:P], rhs=ak_sb[:, cs],
                    start=False, stop=True,
                )
                nc.vector.tensor_copy(qk_sb[2][:, cs], pq2s[ch])

            for ch in range(4):
                cs = slice(ch * 512, (ch + 1) * 512)
                pa = pPro.tile([3 * R, 512], F32, tag=f"pq{ch}", name=f"pa{ch}")
                for kt in range(NKT):
                    nc.tensor.matmul(
                        pa,
                        lhsT=ab_t[kt],
                        rhs=xa_t[kt][:, cs],
                        start=(kt == 0),
                        stop=(kt == NKT - 1),
                    )
                nc.vector.tensor_copy(ak_sb[0:R, cs], pa[0:R, :])
                nc.vector.tensor_copy(av_sb[0:R, cs], pa[2 * R : 3 * R, :])
                if ch >= 1:
                    lora_m2(ch - 1)
            lora_m2(3)

        # Phase D+E: attention units restructured as (head-pair x
        # q-quarter): the two heads' S^T matmuls have K=64 and run
        # CONCURRENTLY in row groups 0-63 / 64-127 (tile_position derives
        # from the qk_sb base partitions), writing the two bank-halves of
        # one (128,1024) PSUM tile -- S stream time halves vs one-head
        # units while the 1024-wide exp and PSUM budget stay identical.
        # Per tj the PE now does ~0.64us (S pair ~0.21 + 2 PV 0.43) vs
        # ACT's 1.15us exp, so every unit has slack to absorb the woven-in
        # B/C/out-proj fillers.  PSUM (8 banks): pS 2x(128,1024)=4,
        # pO 2x(65,512)=2, pX 2x(128,512)=2.
        with (
            tc.tile_pool(name="pS", bufs=2, space="PSUM") as pS,
            tc.tile_pool(name="pO", bufs=2, space="PSUM") as pO,
            tc.tile_pool(name="pX", bufs=2, space="PSUM") as pX,
            tc.tile_pool(name="pP", bufs=3) as pP,
            tc.tile_pool(name="pN", bufs=4) as pN,
            tc.tile_pool(name="pD", bufs=4, space="DRAM") as pD,
            tc.tile_pool(name="pOut", bufs=3) as pOut,
        ):
            def filler_c(mt):
                def f():
                    ms = slice(mt * P, (mt + 1) * P)
                    pv = pX.tile([P, 512], F32, tag="px", name=f"pv_{mt}")
                    for kt in range(NKT):
                        nc.tensor.matmul(
                            pv[:, : HPC * VW],
                            lhsT=xa_t[kt][:, ms],
                            rhs=wv_t[kt],
                            start=(kt == 0),
                            stop=False,
                        )
                    nc.tensor.matmul(
                        pv[:, : HPC * VW], lhsT=av_sb[:, ms], rhs=vb_t,
                        start=False, stop=True,
                    )
                    nc.vector.tensor_copy(v_sb[mt], pv[:, : HPC * VW])
                return f

            def filler_b(m, ch):
                def f():
                    cs = slice(ch * 512, (ch + 1) * 512)
                    pq = pX.tile([P, 512], F32, tag="px", name=f"pq_{m}_{ch}")
                    for kt in range(NKT):
                        nc.tensor.matmul(
                            pq,
                            lhsT=wqk_t[kt][:, m * P : (m + 1) * P],
                            rhs=xa_t[kt][:, cs],
                            start=(kt == 0),
                            stop=(kt == NKT - 1 and m < 2),
                        )
                    if m >= 2:
                        nc.tensor.matmul(
                            pq,
                            lhsT=kb_t[:, (m - 2) * P : (m - 1) * P],
                            rhs=ak_sb[:, cs],
                            start=False,
                            stop=True,
                        )
                        nc.vector.tensor_copy(qk_sb[m][:, cs], pq)
                    else:
                        nc.vector.tensor_scalar_add(
                            qk_sb[m][:, cs], pq, qb_t[:, m : m + 1]
                        )
                return f

            def outproj_mt(mt, act_copies):
                def f():
                    ms = slice(mt * P, (mt + 1) * P)
                    ob = pOut.tile([P, D], BF, tag="ob", name=f"ob_{mt}")
                    for ch in range(2):
                        cs = slice(ch * 512, (ch + 1) * 512)
                        px = pX.tile([P, 512], F32, tag="px", name=f"px_{mt}_{ch}")
                        for kt2 in range(2):
                            nc.tensor.matmul(
                                px,
                                lhsT=oT_sb[kt2][:, ms],
                                rhs=wo_t[kt2][:, cs],
                                start=(kt2 == 0),
                                stop=(kt2 == 1),
                            )
                        if act_copies and ch == 1:
                            nc.scalar.copy(ob[:, cs], px)
                        else:
                            nc.vector.tensor_copy(ob[:, cs], px)
                        nc.sync.dma_start(out=out[ms, cs], in_=ob[:, cs])
                return f

            def emit_unit(pair, qq, fillers, stride, seam=None):
                qTt = qk_sb[pair]
                kTt = qk_sb[2 + pair]
                qs = slice(qq * 512, (qq + 1) * 512)
                pos = [
                    pO.tile([VW, 512], F32, tag="po", name=f"po_{pair}_{qq}_{hh}")
                    for hh in range(2)
                ]
                pts = {}

                def emit_pv(tjp):
                    pt = pts.pop(tjp)
                    for hh in range(2):
                        h = 2 * pair + hh
                        nc.tensor.matmul(
                            pos[hh],
                            lhsT=v_sb[tjp][:, h * VW : (h + 1) * VW],
                            rhs=pt[:, hh * 512 : (hh + 1) * 512],
                            start=(tjp == 0),
                            stop=(tjp == NTT - 1),
                        )

                for tj in range(NTT):
                    ps = pS.tile([P, HF], F32, tag="spsum", name=f"ps_{pair}_{qq}_{tj}")
                    for hh in range(2):
                        nc.tensor.matmul(
                            ps[:, hh * 512 : (hh + 1) * 512],
                            lhsT=kTt[hh * HD : (hh + 1) * HD, tj * P : (tj + 1) * P],
                            rhs=qTt[hh * HD : (hh + 1) * HD, qs],
                            start=True,
                            stop=True,
                        )
                    pt = pP.tile([P, HF], BF, tag="pt", name=f"pt_{pair}_{qq}_{tj}")
                    nc.scalar.activation(pt, ps, mybir.ActivationFunctionType.Exp)
                    pts[tj] = pt
                    if tj == 0 and seam is not None:
                        seam()
                    if fillers and tj % stride == 0:
                        fillers.pop(0)()
                    if tj > 0:
                        emit_pv(tj - 1)
                return pos, lambda: emit_pv(NTT - 1)

            def emit_norm_copy(pair, qq, hh, po):
                un = pN.tile([VW, 512], F32, tag="un", name=f"un_{pair}_{qq}_{hh}")
                nc.vector.tensor_copy(un, po)
                dr = pD.tile([1, 512], F32, tag="dr", name=f"dr_{pair}_{qq}_{hh}")
                nc.sync.dma_start(out=dr, in_=un[HD:VW, :])
                rs = pN.tile([HD, 8], F32, tag="rs", name=f"rs_{pair}_{qq}_{hh}")
                nc.sync.dma_start(
                    out=rs,
                    in_=bass.AP(tensor=dr.tensor, offset=dr.offset, ap=[[8, HD], [1, 8]]),
                )
                rr = pN.tile([HD, 8], F32, tag="rr", name=f"rr_{pair}_{qq}_{hh}")
                nc.vector.reciprocal(rr, rs)
                dr2 = pD.tile([HD, 8], F32, tag="dr2", name=f"dr2_{pair}_{qq}_{hh}")
                nc.sync.dma_start(out=dr2, in_=rr)
                rec = pN.tile([HD, 512], F32, tag="rec", name=f"rec_{pair}_{qq}_{hh}")
                nc.sync.dma_start(
                    out=rec,
                    in_=bass.AP(tensor=dr2.tensor, offset=dr2.offset, ap=[[0, HD], [1, 512]]),
                )
                return un, rec

            def emit_norm_finish(pair, qq, hh, un, rec):
                nc.vector.tensor_mul(
                    oT_sb[pair][hh * HD : (hh + 1) * HD, qq * 512 : (qq + 1) * 512],
                    un[0:HD, :],
                    rec,
                )

            def emit_tail(pair, qq, pos):
                # Last unit's two heads normalized via 1/Z = exp(-ln Z) on
                # the now-idle ACT (same table set as exp) + a K=1
                # ones-column matmul broadcast -- no DMA on the tail path;
                # dummy matmuls off the chain tiles keep HAM warm.
                for hh in range(2):
                    po = pos[hh]
                    un = pN.tile([VW, 512], F32, tag="unc", name=f"unc_{hh}")
                    nc.vector.tensor_copy(un, po)
                    lnz = pN.tile([1, 512], F32, tag="lnz", name=f"lnz_{hh}")
                    nc.scalar.activation(
                        lnz, un[HD:VW, :], mybir.ActivationFunctionType.Ln
                    )
                    rcp = pN.tile([1, 512], F32, tag="rcp", name=f"rcp_{hh}")
                    nc.scalar.activation(
                        rcp, lnz, mybir.ActivationFunctionType.Exp, scale=-1.0
                    )
                    dm = pS.tile([P, HF], F32, tag="spsum", name=f"dm_a{hh}")
                    nc.tensor.matmul(
                        dm[:, 0:64], lhsT=un[0:HD, 0:P], rhs=un[0:HD, 0:64],
                        start=True, stop=True,
                    )
                    rec = pX.tile([P, 512], F32, tag="px", name=f"recp_{hh}")
                    nc.tensor.matmul(
                        rec[0:HD, :], lhsT=ones_t, rhs=rcp, start=True, stop=True
                    )
                    nc.vector.tensor_mul(
                        oT_sb[pair][hh * HD : (hh + 1) * HD, qq * 512 : (qq + 1) * 512],
                        un[0:HD, :],
                        rec[0:HD, :],
                    )
                for k, mt in enumerate(range(12, 16)):
                    outproj_mt(mt, act_copies=(k % 2 == 0))()

            # Units: pair-01 quarters then pair-23 quarters.  C mt j must be
            # emitted by unit-0 slot j (PV consumes v_sb[j]); B m1/m3 must
            # finish before unit 4 (pair 23) -> spread over units 1-3;
            # out-proj group g (mt 4g..4g+3) unlocks when unit 4+g's norm
            # finishes (seam of unit 6+g) -> groups 0,1 woven into units
            # 6,7 as fillers, group 2 post-loop, group 3 after the tail.
            fillers_u = [[] for _ in range(8)]
            fillers_u[0] = [filler_c(mt) for mt in range(NTT)]
            bl = [filler_b(m, ch) for ch in range(4) for m in (1, 3)]
            fillers_u[1] = bl[0:3]
            fillers_u[2] = bl[3:6]
            fillers_u[3] = bl[6:8]
            strides = [1, 5, 5, 5, 4, 4, 4, 4]

            # bridge the ~1.7us dep-wait before unit 0's first S matmul
            # (m2-ch0 copy in flight) so HAM doesn't re-throttle right at
            # attention start
            dmw = pS.tile([P, HF], F32, tag="spsum", name="dmw")
            for _ in range(10):
                nc.tensor.matmul(
                    dmw[0:HD, 0:HD], lhsT=ones_t, rhs=ones_t, start=True, stop=True
                )

            units = [(pair, qq) for pair in range(2) for qq in range(4)]
            state = {"prev": None, "tofinish": None}

            def make_seam(i):
                def seam():
                    p = state["prev"]
                    if p is None:
                        return
                    p["final_pv"]()
                    cur = None
                    if p["idx"] < 7:
                        cur = [
                            emit_norm_copy(p["pair"], p["qq"], hh, p["pos"][hh])
                            for hh in range(2)
                        ]
                    f = state["tofinish"]
                    if f is not None:
                        for args in f:
                            emit_norm_finish(*args)
                        g = i - 6
                        if g >= 0:
                            for j in range(4):
                                fillers_u[i].append(
                                    outproj_mt(4 * g + j, act_copies=False)
                                )
                    state["tofinish"] = (
                        [
                            (p["pair"], p["qq"], hh, cur[hh][0], cur[hh][1])
                            for hh in range(2)
                        ]
                        if cur
                        else None
                    )
                return seam

            for i, (pair, qq) in enumerate(units):
                pos, fpv = emit_unit(
                    pair, qq, fillers_u[i], strides[i], make_seam(i)
                )
                state["prev"] = {
                    "idx": i, "pair": pair, "qq": qq, "pos": pos, "final_pv": fpv,
                }
            state["prev"]["final_pv"]()
            for args in state["tofinish"]:
                emit_norm_finish(*args)
            for j in range(8, 12):
                outproj_mt(j, act_copies=False)()
            emit_tail(1, 3, state["prev"]["pos"])

    # bass.Bass's finalize skips Bacc's wait-splitting passes; walrus allows
    # at most 1 sync wait per instruction (2 for event semaphores), so run
    # just those two passes here.
    import bass_rust as _bass_rust

    _bass_rust.move_matmul_waits_to_ldweights(nc.m)
    _bass_rust.generate_event_semaphores(nc)
    return nc


def prepare_in_maps(inputs):
    q = np.asarray(inputs["query"], np.float32)
    ipw = np.asarray(inputs["in_proj_weight"], np.float32)
    ipb = np.asarray(inputs["in_proj_bias"], np.float32)
    out_w = np.asarray(inputs["out_w"], np.float32)
    k_a = np.asarray(inputs["k_a"], np.float32)
    k_b = np.asarray(inputs["k_b"], np.float32)
    v_a = np.asarray(inputs["v_a"], np.float32)
    v_b = np.asarray(inputs["v_b"], np.float32)
    qscale = 1.0 / math.sqrt(HD)
    sl = SCALE / R

    in_maps = []
    for c in range(NCORES):
        bb = c // 4
        s = (c % 4) * CD
        e = s + CD
        X = q[:, bb, :]

        xa = X.T  # (D, T)

        wqk = np.zeros((D, 2 * CD), np.float32)
        wqk[:, :CD] = ipw[s:e].T * qscale
        wqk[:, CD:] = ipw[D + s : D + e].T

        wv = np.zeros((D, HPC * VW), np.float32)
        for j in range(HPC):
            wv[:, j * VW : j * VW + HD] = ipw[2 * D + s + j * HD : 2 * D + s + (j + 1) * HD].T
        # pack 8 k-tiles column-wise (see kernel comment on DMA descriptors)
        wv = wv.reshape(NKT, P, HPC * VW).transpose(1, 0, 2).reshape(P, NKT * HPC * VW)

        ab = np.zeros((D, 3 * R), np.float32)
        ab[:, :R] = k_a.T
        ab[:, 2 * R :] = v_a.T
        ab = ab.reshape(NKT, P, 3 * R).transpose(1, 0, 2).reshape(P, NKT * 3 * R)

        kbm = np.zeros((RA, CD), np.float32)
        kbm[:R] = k_b[:, s:e] * sl
        kbm[R] = ipb[D + s : D + e]  # K bias via ones row

        vbm = np.zeros((RA, HPC * VW), np.float32)
        for j in range(HPC):
            vbm[:R, j * VW : j * VW + HD] = v_b[:, s + j * HD : s + (j + 1) * HD] * sl
            vbm[R, j * VW : j * VW + HD] = ipb[2 * D + s + j * HD : 2 * D + s + (j + 1) * HD]
            vbm[R, j * VW + HD] = 1.0  # denominator ones column

        qbias = np.stack([ipb[s : s + P], ipb[s + P : s + 2 * P]], axis=1) * qscale

        wo = out_w[:, s:e].T

        in_maps.append(
            {
                "xa": xa.astype(BF16),
                "wqk": wqk.astype(BF16),
                "wv": wv.astype(BF16),
                "ab": ab.astype(BF16),
                "kbm": kbm.astype(BF16),
                "vbm": vbm.astype(BF16),
                "qb": qbias.astype(np.float32),
                "wo": wo.astype(BF16),
            }
        )
    return in_maps


def assemble_output(inputs, results):
    out_b = np.asarray(inputs["out_b"], np.float32)
    out = np.zeros((T, BSZ, D), np.float32)
    for c in range(NCORES):
        out[:, c // 4, :] += results[c]["out"].astype(np.float32)
    out += out_b[None, None, :]
    return out


def kernel(**inputs):
    nc = build_nc()
    in_maps = prepare_in_maps(inputs)
    res = run_bass_kernel_spmd(nc, in_maps, core_ids=list(range(NCORES)))
    return assemble_output(inputs, res.results)


# revision 43
# speedup vs baseline: 1.1799x; 1.0392x over previous
"""LoRA MultiheadAttention on 8 NeuronCores (Bass/Tile).

Sharding: 32 (batch, head) attention slices -> 4 heads x 1 batch per core.
Cores 0-3 take batch 0, cores 4-7 batch 1; core c handles heads
(c%4)*4 .. (c%4)*4+3, i.e. a contiguous 256-wide slice of the head dims.

Per-core math (all big matmuls bf16 on PE, fp32 PSUM accumulate):
  qkT   = wqk^T-slices @ X  -> Q^T, K^T in (head-dim, T) layout
          (q pre-scaled by 1/sqrt(hd)); Q bias added as a per-partition
          tensor_scalar during the PSUM->SBUF copy, K bias folded into a
          17th row of the LoRA-K accumulation matmul (ones row in A^T)
  V     = X @ Wv-slice, per-head 65-wide blocks with a ones column ->
          the PV matmul emits the softmax denominator for free; V bias +
          the ones-column constant folded into a 17th LoRA-V row
  S^T   = K^T.T-slices @ Q^T  (tj on partitions, ti free)  [K=64].
          Units are (head-pair x q-quarter): the pair's two K=64 matmuls
          run concurrently in PE row groups 0-63/64-127 (tile_position),
          halving S stream time into the two halves of one (128,1024)
          PSUM tile
  P^T   = exp(S^T) on ACT, 1024-wide tiles (no max-subtraction: |s|<~3)
  O^T   = V_aug.T @ P^T accumulated over tj; row 64 = denom
  norm  : units 0-6: denom row -> DRAM -> (64,16) reshape -> cheap
          InstReciprocal -> DRAM -> (64,1024) stride-0 broadcast ->
          multiply (split copy/finish so the single po bank pair frees
          before the next unit's PV).  Last unit: two 512-wide chains
          with 1/Z = exp(-ln Z) on the then-idle ACT (same table set as
          exp) and a K=1 ones-column matmul broadcast -- no DMA hops on
          the critical tail path.
  out   = O^T.T @ out_w-slice^T, bf16 partials summed on host.

The attention phase is ACT-exp-bound (16.8M exps/core = 109us floor
at 1 elem/lane/cycle @1.2GHz); with pair-packed S the PE has slack in
every unit, so Phase B(m1,m3) + all of Phase C + most of the out-proj
are woven into the attention units' PE stream as fillers: ACT runs
continuously while the PE never gaps >3.4us (keeps HAM at 2.4 GHz).

out_b added on host.
"""

import sys

sys.path.insert(0, "/opt/trn_rl_repo")

import math
from contextlib import ExitStack

import ml_dtypes
import numpy as np

import concourse.bass as bass
import concourse.tile as tile
from concourse import bacc
from concourse import mybir
from concourse import bass_utils as _bu
from concourse.bass_utils import run_bass_kernel_spmd

# (note: walrus's --enable-ldw-opt is hardcoded false for a reason --
# bass-emitted InstLdweights is rejected by that pass.)

BF16 = ml_dtypes.bfloat16
F32 = mybir.dt.float32
BF = mybir.dt.bfloat16

T = 2048
D = 1024
H = 16
HD = 64
R = 16
RA = R + 1  # LoRA rank + ones row (bias folding)
BSZ = 2
SCALE = 16.0
NCORES = 8
HPC = 4  # heads per core
CD = HPC * HD  # 256 head dims per core
VW = HD + 1  # V block width per head (ones column appended)
NKT = D // 128  # 8 contraction k-tiles (no bias row: biases are folded)
P = 128
NTT = T // P  # 16 row tiles
HF = T // 2  # 1024: ti processed in two halves


def build_nc():
    nc = bass.Bass()
    xa = nc.dram_tensor("xa", [D, T], BF, kind="ExternalInput")
    wqk = nc.dram_tensor("wqk", [D, 2 * CD], BF, kind="ExternalInput")
    wv = nc.dram_tensor("wv", [P, NKT * HPC * VW], BF, kind="ExternalInput")
    ab = nc.dram_tensor("ab", [P, NKT * 3 * R], BF, kind="ExternalInput")
    kbm = nc.dram_tensor("kbm", [RA, CD], BF, kind="ExternalInput")
    vbm = nc.dram_tensor("vbm", [RA, HPC * VW], BF, kind="ExternalInput")
    qb = nc.dram_tensor("qb", [P, 2], F32, kind="ExternalInput")
    wo = nc.dram_tensor("wo", [CD, D], BF, kind="ExternalInput")
    out = nc.dram_tensor("out", [T, D], BF, kind="ExternalOutput")

    with tile.TileContext(nc) as tc, ExitStack() as ctx:
        singles = ctx.enter_context(tc.tile_pool(name="singles", bufs=1))

        xa_t = [singles.tile([P, T], BF, name=f"xa{i}", tag=f"xa{i}") for i in range(NKT)]
        wqk_t = [singles.tile([P, 2 * CD], BF, name=f"wqk{i}", tag=f"wqk{i}") for i in range(NKT)]
        # ab/wv k-tiles packed column-wise into one tile each: their
        # natural per-tile partition rows are 96/520 bytes, which makes the
        # DMA descriptor-rate-bound (1024 tiny descriptors held Phase A's
        # weights hostage for ~12us).  Packed, the rows are 8x larger.
        wvp = singles.tile([P, NKT * HPC * VW], BF, tag="wvp")
        abp = singles.tile([P, NKT * 3 * R], BF, tag="abp")
        wv_t = [wvp[:, i * HPC * VW : (i + 1) * HPC * VW] for i in range(NKT)]
        ab_t = [abp[:, i * 3 * R : (i + 1) * 3 * R] for i in range(NKT)]
        kb_t = singles.tile([RA, CD], BF, tag="kb")
        vb_t = singles.tile([RA, HPC * VW], BF, tag="vb")
        qb_t = singles.tile([P, 2], F32, tag="qb")
        wo_t = [singles.tile([P, D], BF, name=f"wo{i}", tag=f"wo{i}") for i in range(2)]
        # DMA issue order matches consumption: A needs ab+xa, then B needs
        # wqk, C (interleaved into unit 0) needs wv, out-proj needs wo last.
        # Input DMAs spread across three engine queues: ~600ns of submit
        # cost per DMA serializes on a single queue (27 submits = 16us
        # before the last transfer even starts), and the transfers
        # themselves overlap across hardware queues.  Order within each
        # queue matches consumption: A needs ab+xa first, B m0 needs wqk
        # ~10us in, C fillers need wv by ~unit 0, wo last.
        for i in range(0, NKT, 2):
            nc.sync.dma_start(out=xa_t[i], in_=xa[i * P : (i + 1) * P, :])
            nc.scalar.dma_start(out=xa_t[i + 1], in_=xa[(i + 1) * P : (i + 2) * P, :])
            nc.scalar.dma_start(out=wqk_t[i], in_=wqk[i * P : (i + 1) * P, :])
            nc.sync.dma_start(out=wqk_t[i + 1], in_=wqk[(i + 1) * P : (i + 2) * P, :])
        nc.sync.dma_start(out=qb_t, in_=qb[:, :])
        nc.sync.dma_start(out=kb_t, in_=kbm[:, :])
        nc.scalar.dma_start(out=abp, in_=ab[:, :])
        nc.scalar.dma_start(out=vb_t, in_=vbm[:, :])
        nc.sync.dma_start(out=wvp, in_=wv[:, :])
        for i in range(2):
            nc.scalar.dma_start(out=wo_t[i], in_=wo[i * P : (i + 1) * P, :])

        ones_t = singles.tile([1, HD], F32, tag="ones")
        nc.vector.memset(ones_t, 1.0)

        # Dummy exp with no deps: walrus's ACT_TABLE_LOAD for the exp set
        # (~2.7us) runs during the input DMA wait instead of at the first
        # real attention exp.
        scr_t = singles.tile([1, HD], BF, tag="scr")
        nc.scalar.activation(scr_t, ones_t, mybir.ActivationFunctionType.Exp)

        qk_sb = [singles.tile([P, T], BF, name=f"qk{i}", tag=f"qk{i}") for i in range(4)]
        ak_sb = singles.tile([RA, T], BF, tag="ak")
        av_sb = singles.tile([RA, T], BF, tag="av")
        v_sb = [singles.tile([P, HPC * VW], BF, name=f"v{i}", tag=f"v{i}") for i in range(NTT)]
        oT_sb = [singles.tile([P, T], BF, name=f"oT{i}", tag=f"oT{i}") for i in range(2)]

        # ones row for the bias-folding contraction (row 16 of A^T tiles):
        # engines can't address a 1-partition region at base 16, so memset
        # the whole tile and let Phase A overwrite rows 0-15.
        nc.vector.memset(ak_sb, 1.0)
        nc.vector.memset(av_sb, 1.0)

        # Prologue: B m0 and B m2's main K=1024 contractions ride one
        # kt-outer loop (8 live accumulators = all 8 PSUM banks), paced by
        # the two DMA queues delivering xa[kt]+wqk[kt].  Phase A then runs
        # on the 4 banks freed by the m0 copies (same-tag tile reuse), and
        # the LoRA-K + K-bias matmul accumulates into the still-live m2
        # banks before their copies.  Serial pre-attention work after the
        # input stream shrinks to A + lora + copies (~9us).
        with tc.tile_pool(name="pPro", bufs=1, space="PSUM") as pPro:
            pqs = [
                pPro.tile([P, 512], F32, tag=f"pq{ch}", name=f"pq0{ch}")
                for ch in range(4)
            ]
            pq2s = [
                pPro.tile([P, 512], F32, tag=f"pq2{ch}", name=f"pq2{ch}")
                for ch in range(4)
            ]
            for kt in range(NKT):
                for ch in range(4):
                    nc.tensor.matmul(
                        pqs[ch],
                        lhsT=wqk_t[kt][:, 0:P],
                        rhs=xa_t[kt][:, ch * 512 : (ch + 1) * 512],
                        start=(kt == 0),
                        stop=(kt == NKT - 1),
                    )
                for ch in range(4):
                    nc.tensor.matmul(
                        pq2s[ch],
                        lhsT=wqk_t[kt][:, 2 * P : 3 * P],
                        rhs=xa_t[kt][:, ch * 512 : (ch + 1) * 512],
                        start=(kt == 0),
                        stop=False,
                    )
            for ch in range(4):
                cs = slice(ch * 512, (ch + 1) * 512)
                nc.vector.tensor_scalar_add(qk_sb[0][:, cs], pqs[ch], qb_t[:, 0:1])
            # Phase A on the freed m0 banks, with the LoRA-K + K-bias
            # matmul and m2 copy of chunk ch-1 interleaved after A's chunk
            # ch so the PE reaches the first attention S matmul with its
            # qk_sb[2] dependency already satisfied (no end-of-prologue
            # bubble, no HAM re-throttle at attention start).
            def lora_m2(ch):
                cs = slice(ch * 512, (ch + 1) * 512)
                nc.tensor.matmul(
                    pq2s[ch], lhsT=kb_t[:, 0:P], rhs=ak_sb[:, cs],
                    start=False, stop=True,
                )
                nc.vector.tensor_copy(qk_sb[2][:, cs], pq2s[ch])

            for ch in range(4):
                cs = slice(ch * 512, (ch + 1) * 512)
                pa = pPro.tile([3 * R, 512], F32, tag=f"pq{ch}", name=f"pa{ch}")
                for kt in range(NKT):
                    nc.tensor.matmul(
                        pa,
                        lhsT=ab_t[kt],
                        rhs=xa_t[kt][:, cs],
                        start=(kt == 0),
                        stop=(kt == NKT - 1),
                    )
                nc.vector.tensor_copy(ak_sb[0:R, cs], pa[0:R, :])
                nc.vector.tensor_copy(av_sb[0:R, cs], pa[2 * R : 3 * R, :])
                if ch >= 1:
                    lora_m2(ch - 1)
            lora_m2(3)

        # Phase D+E: attention units restructured as (head-pair x
        # q-quarter): the two heads' S^T matmuls have K=64 and run
        # CONCURRENTLY in row groups 0-63 / 64-127 (tile_position derives
        # from the qk_sb base partitions), writing the two bank-halves of
        # one (128,1024) PSUM tile -- S stream time halves vs one-head
        # units while the 1024-wide exp and PSUM budget stay identical.
        # Per tj the PE now does ~0.64us (S pair ~0.21 + 2 PV 0.43) vs
        # ACT's 1.15us exp, so every unit has slack to absorb the woven-in
        # B/C/out-proj fillers.  PSUM (8 banks): pS 2x(128,1024)=4,
        # pO 2x(65,512)=2, pX 2x(128,512)=2.
        with (
            tc.tile_pool(name="pS", bufs=2, space="PSUM") as pS,
            tc.tile_pool(name="pO", bufs=2, space="PSUM") as pO,
            tc.tile_pool(name="pX", bufs=2, space="PSUM") as pX,
            tc.tile_pool(name="pP", bufs=3) as pP,
            tc.tile_pool(name="pN", bufs=4) as pN,
            tc.tile_pool(name="pD", bufs=4, space="DRAM") as pD,
            tc.tile_pool(name="pOut", bufs=3) as pOut,
        ):
            def filler_c(mt):
                def f():
                    ms = slice(mt * P, (mt + 1) * P)
                    pv = pX.tile([P, 512], F32, tag="px", name=f"pv_{mt}")
                    for kt in range(NKT):
                        nc.tensor.matmul(
                            pv[:, : HPC * VW],
                            lhsT=xa_t[kt][:, ms],
                            rhs=wv_t[kt],
                            start=(kt == 0),
                            stop=False,
                        )
                    nc.tensor.matmul(
                        pv[:, : HPC * VW], lhsT=av_sb[:, ms], rhs=vb_t,
                        start=False, stop=True,
                    )
                    nc.vector.tensor_copy(v_sb[mt], pv[:, : HPC * VW])
                return f

            def filler_b(m, ch):
                def f():
                    cs = slice(ch * 512, (ch + 1) * 512)
                    pq = pX.tile([P, 512], F32, tag="px", name=f"pq_{m}_{ch}")
                    for kt in range(NKT):
                        nc.tensor.matmul(
                            pq,
                            lhsT=wqk_t[kt][:, m * P : (m + 1) * P],
                            rhs=xa_t[kt][:, cs],
                            start=(kt == 0),
                            stop=(kt == NKT - 1 and m < 2),
                        )
                    if m >= 2:
                        nc.tensor.matmul(
                            pq,
                            lhsT=kb_t[:, (m - 2) * P : (m - 1) * P],
                            rhs=ak_sb[:, cs],
                            start=False,
                            stop=True,
                        )
                        nc.vector.tensor_copy(qk_sb[m][:, cs], pq)
                    else:
                        nc.vector.tensor_scalar_add(
                            qk_sb[m][:, cs], pq, qb_t[:, m : m + 1]
                        )
                return f

            def outproj_mt(mt, act_copies):
                def f():
                    ms = slice(mt * P, (mt + 1) * P)
                    ob = pOut.tile([P, D], BF, tag="ob", name=f"ob_{mt}")
                    for ch in range(2):
                        cs = slice(ch * 512, (ch + 1) * 512)
                        px = pX.tile([P, 512], F32, tag="px", name=f"px_{mt}_{ch}")
                        for kt2 in range(2):
                            nc.tensor.matmul(
                                px,
                                lhsT=oT_sb[kt2][:, ms],
                                rhs=wo_t[kt2][:, cs],
                                start=(kt2 == 0),
                                stop=(kt2 == 1),
                            )
                        if act_copies and ch == 1:
                            nc.scalar.copy(ob[:, cs], px)
                        else:
                            nc.vector.tensor_copy(ob[:, cs], px)
                        nc.sync.dma_start(out=out[ms, cs], in_=ob[:, cs])
                return f

            def emit_unit(pair, qq, fillers, stride, seam=None):
                qTt = qk_sb[pair]
                kTt = qk_sb[2 + pair]
                qs = slice(qq * 512, (qq + 1) * 512)
                pos = [
                    pO.tile([VW, 512], F32, tag="po", name=f"po_{pair}_{qq}_{hh}")
                    for hh in range(2)
                ]
                pts = {}

                def emit_pv(tjp):
                    pt = pts.pop(tjp)
                    for hh in range(2):
                        h = 2 * pair + hh
                        nc.tensor.matmul(
                            pos[hh],
                            lhsT=v_sb[tjp][:, h * VW : (h + 1) * VW],
                            rhs=pt[:, hh * 512 : (hh + 1) * 512],
                            start=(tjp == 0),
                            stop=(tjp == NTT - 1),
                        )

                for tj in range(NTT):
                    ps = pS.tile([P, HF], F32, tag="spsum", name=f"ps_{pair}_{qq}_{tj}")
                    for hh in range(2):
                        nc.tensor.matmul(
                            ps[:, hh * 512 : (hh + 1) * 512],
                            lhsT=kTt[hh * HD : (hh + 1) * HD, tj * P : (tj + 1) * P],
                            rhs=qTt[hh * HD : (hh + 1) * HD, qs],
                            start=True,
                            stop=True,
                        )
                    pt = pP.tile([P, HF], BF, tag="pt", name=f"pt_{pair}_{qq}_{tj}")
                    nc.scalar.activation(pt, ps, mybir.ActivationFunctionType.Exp)
                    pts[tj] = pt
                    if tj == 0 and seam is not None:
                        seam()
                    if fillers and tj % stride == 0:
                        fillers.pop(0)()
                    if tj > 0:
                        emit_pv(tj - 1)
                return pos, lambda: emit_pv(NTT - 1)

            def emit_norm_copy(pair, qq, hh, po):
                un = pN.tile([VW, 512], F32, tag="un", name=f"un_{pair}_{qq}_{hh}")
                nc.vector.tensor_copy(un, po)
                dr = pD.tile([1, 512], F32, tag="dr", name=f"dr_{pair}_{qq}_{hh}")
                nc.sync.dma_start(out=dr, in_=un[HD:VW, :])
                rs = pN.tile([HD, 8], F32, tag="rs", name=f"rs_{pair}_{qq}_{hh}")
                nc.sync.dma_start(
                    out=rs,
                    in_=bass.AP(tensor=dr.tensor, offset=dr.offset, ap=[[8, HD], [1, 8]]),
                )
                rr = pN.tile([HD, 8], F32, tag="rr", name=f"rr_{pair}_{qq}_{hh}")
                nc.vector.reciprocal(rr, rs)
                dr2 = pD.tile([HD, 8], F32, tag="dr2", name=f"dr2_{pair}_{qq}_{hh}")
                nc.sync.dma_start(out=dr2, in_=rr)
                rec = pN.tile([HD, 512], F32, tag="rec", name=f"rec_{pair}_{qq}_{hh}")
                nc.sync.dma_start(
                    out=rec,
                    in_=bass.AP(tensor=dr2.tensor, offset=dr2.offset, ap=[[0, HD], [1, 512]]),
                )
                return un, rec

            def emit_norm_finish(pair, qq, hh, un, rec):
                nc.vector.tensor_mul(
                    oT_sb[pair][hh * HD : (hh + 1) * HD, qq * 512 : (qq + 1) * 512],
                    un[0:HD, :],
                    rec,
                )

            def emit_tail(pair, qq, pos):
                # Last unit's two heads normalized via 1/Z = exp(-ln Z) on
                # the now-idle ACT (same table set as exp) + a K=1
                # ones-column matmul broadcast -- no DMA on the tail path;
                # dummy matmuls off the chain tiles keep HAM warm.
                for hh in range(2):
                    po = pos[hh]
                    un = pN.tile([VW, 512], F32, tag="unc", name=f"unc_{hh}")
                    nc.vector.tensor_copy(un, po)
                    lnz = pN.tile([1, 512], F32, tag="lnz", name=f"lnz_{hh}")
                    nc.scalar.activation(
                        lnz, un[HD:VW, :], mybir.ActivationFunctionType.Ln
                    )
                    rcp = pN.tile([1, 512], F32, tag="rcp", name=f"rcp_{hh}")
                    nc.scalar.activation(
                        rcp, lnz, mybir.ActivationFunctionType.Exp, scale=-1.0
                    )
                    dm = pS.tile([P, HF], F32, tag="spsum", name=f"dm_a{hh}")
                    nc.tensor.matmul(
                        dm[:, 0:64], lhsT=un[0:HD, 0:P], rhs=un[0:HD, 0:64],
                        start=True, stop=True,
                    )
                    rec = pX.tile([P, 512], F32, tag="px", name=f"recp_{hh}")
                    nc.tensor.matmul(
                        rec[0:HD, :], lhsT=ones_t, rhs=rcp, start=True, stop=True
                    )
                    nc.vector.tensor_mul(
                        oT_sb[pair][hh * HD : (hh + 1) * HD, qq * 512 : (qq + 1) * 512],
                        un[0:HD, :],
                        rec[0:HD, :],
                    )
                for k, mt in enumerate(range(12, 16)):
                    outproj_mt(mt, act_copies=(k % 2 == 0))()

            # Units: pair-01 quarters then pair-23 quarters.  C mt j must be
            # emitted by unit-0 slot j (PV consumes v_sb[j]); B m1/m3 must
            # finish before unit 4 (pair 23) -> spread over units 1-3;
            # out-proj group g (mt 4g..4g+3) unlocks when unit 4+g's norm
            # finishes (seam of unit 6+g) -> groups 0,1 woven into units
            # 6,7 as fillers, group 2 post-loop, group 3 after the tail.
            fillers_u = [[] for _ in range(8)]
            fillers_u[0] = [filler_c(mt) for mt in range(NTT)]
            bl = [filler_b(m, ch) for ch in range(4) for m in (1, 3)]
            fillers_u[1] = bl[0:3]
            fillers_u[2] = bl[3:6]
            fillers_u[3] = bl[6:8]
            strides = [1, 5, 5, 5, 4, 4, 4, 4]

            units = [(pair, qq) for pair in range(2) for qq in range(4)]
            state = {"prev": None, "tofinish": None}

            def make_seam(i):
                def seam():
                    p = state["prev"]
                    if p is None:
                        return
                    p["final_pv"]()
                    cur = None
                    if p["idx"] < 7:
                        cur = [
                            emit_norm_copy(p["pair"], p["qq"], hh, p["pos"][hh])
                            for hh in range(2)
                        ]
                    f = state["tofinish"]
                    if f is not None:
                        for args in f:
                            emit_norm_finish(*args)
                        g = i - 6
                        if g >= 0:
                            for j in range(4):
                                fillers_u[i].append(
                                    outproj_mt(4 * g + j, act_copies=False)
                                )
                    state["tofinish"] = (
                        [
                            (p["pair"], p["qq"], hh, cur[hh][0], cur[hh][1])
                            for hh in range(2)
                        ]
                        if cur
                        else None
                    )
                return seam

            for i, (pair, qq) in enumerate(units):
                pos, fpv = emit_unit(
                    pair, qq, fillers_u[i], strides[i], make_seam(i)
                )
                state["prev"] = {
                    "idx": i, "pair": pair, "qq": qq, "pos": pos, "final_pv": fpv,
                }
            state["prev"]["final_pv"]()
            for args in state["tofinish"]:
                emit_norm_finish(*args)
            for j in range(8, 12):
                outproj_mt(j, act_copies=False)()
            emit_tail(1, 3, state["prev"]["pos"])

    # bass.Bass's finalize skips Bacc's wait-splitting passes; walrus allows
    # at most 1 sync wait per instruction (2 for event semaphores), so run
    # just those two passes here.
    import bass_rust as _bass_rust

    _bass_rust.move_matmul_waits_to_ldweights(nc.m)
    _bass_rust.generate_event_semaphores(nc)
    return nc


def prepare_in_maps(inputs):
    q = np.asarray(inputs["query"], np.float32)
    ipw = np.asarray(inputs["in_proj_weight"], np.float32)
    ipb = np.asarray(inputs["in_proj_bias"], np.float32)
    out_w = np.asarray(inputs["out_w"], np.float32)
    k_a = np.asarray(inputs["k_a"], np.float32)
    k_b = np.asarray(inputs["k_b"], np.float32)
    v_a = np.asarray(inputs["v_a"], np.float32)
    v_b = np.asarray(inputs["v_b"], np.float32)
    qscale = 1.0 / math.sqrt(HD)
    sl = SCALE / R

    in_maps = []
    for c in range(NCORES):
        bb = c // 4
        s = (c % 4) * CD
        e = s + CD
        X = q[:, bb, :]

        xa = X.T  # (D, T)

        wqk = np.zeros((D, 2 * CD), np.float32)
        wqk[:, :CD] = ipw[s:e].T * qscale
        wqk[:, CD:] = ipw[D + s : D + e].T

        wv = np.zeros((D, HPC * VW), np.float32)
        for j in range(HPC):
            wv[:, j * VW : j * VW + HD] = ipw[2 * D + s + j * HD : 2 * D + s + (j + 1) * HD].T
        # pack 8 k-tiles column-wise (see kernel comment on DMA descriptors)
        wv = wv.reshape(NKT, P, HPC * VW).transpose(1, 0, 2).reshape(P, NKT * HPC * VW)

        ab = np.zeros((D, 3 * R), np.float32)
        ab[:, :R] = k_a.T
        ab[:, 2 * R :] = v_a.T
        ab = ab.reshape(NKT, P, 3 * R).transpose(1, 0, 2).reshape(P, NKT * 3 * R)

        kbm = np.zeros((RA, CD), np.float32)
        kbm[:R] = k_b[:, s:e] * sl
        kbm[R] = ipb[D + s : D + e]  # K bias via ones row

        vbm = np.zeros((RA, HPC * VW), np.float32)
        for j in range(HPC):
            vbm[:R, j * VW : j * VW + HD] = v_b[:, s + j * HD : s + (j + 1) * HD] * sl
            vbm[R, j * VW : j * VW + HD] = ipb[2 * D + s + j * HD : 2 * D + s + (j + 1) * HD]
            vbm[R, j * VW + HD] = 1.0  # denominator ones column

        qbias = np.stack([ipb[s : s + P], ipb[s + P : s + 2 * P]], axis=1) * qscale

        wo = out_w[:, s:e].T

        in_maps.append(
            {
                "xa": xa.astype(BF16),
                "wqk": wqk.astype(BF16),
                "wv": wv.astype(BF16),
                "ab": ab.astype(BF16),
                "kbm": kbm.astype(BF16),
                "vbm": vbm.astype(BF16),
                "qb": qbias.astype(np.float32),
                "wo": wo.astype(BF16),
            }
        )
    return in_maps


def assemble_output(inputs, results):
    out_b = np.asarray(inputs["out_b"], np.float32)
    out = np.zeros((T, BSZ, D), np.float32)
    for c in range(NCORES):
        out[:, c // 4, :] += results[c]["out"].astype(np.float32)
    out += out_b[None, None, :]
    return out


def kernel(**inputs):
    nc = build_nc()
    in_maps = prepare_in_maps(inputs)
    res = run_bass_kernel_spmd(nc, in_maps, core_ids=list(range(NCORES)))
    return assemble_output(inputs, res.results)
